# revision 2
# baseline (speedup 1.0000x reference)
"""Multi-head attention (B=8, N=1024, H=12, D=64, C=768) on 8 trn2 cores.

Sharding: data-parallel over batch (core b owns x[b]; weights replicated;
no collectives). All matmul operands bf16 (host-cast); PSUM accumulates fp32.

Per-core dataflow:
  warmup:  dummy matmuls ramp the PE clock during the input-DMA window
  phase 1a: qT_t/kT_t [128 x N] = Wqk_block.T @ x^T    (d-major, pair t)
  phase 1b: v_aug[m]  [128 x 12 x 65] = x_m @ Wv (+ ones col per head)
  phase 2 (per head pair t, n-half nh, m-tile):
     s_ps[m,n]   = K^T.T @ Q^T    (two K=64 matmuls, one per head j)
     p_sb        = exp(s_ps/8)    (one [128,1024] ScalarE op, bf16 out)
     acc[n,d+1] += p_chunk.T @ v_aug  (lhsT=P^T chunk, N=65; col 64 = rowsum)
     normalize:   one strided reciprocal + one broadcast tensor_mul -> h_sb
  transpose(t): h_sb[t] [n, hid] -> hT[t] [hid, n] via DMA xbar transpose
  phase 3: y[m] = hT.T @ W_proj; m=0..3 interleaved into attention(5) nh1

Emission interleaves v-tile / next-pair qk / proj matmuls into the attention
m-loop (filler deque) so PE stays fed during exp latency.
"""
from collections import deque

import numpy as np

import concourse.bass as bass
import concourse.mybir as mybir
import concourse.tile as tile
from concourse import bacc
from concourse.bass_utils import run_bass_kernel_spmd

BF16 = mybir.dt.bfloat16
F32 = mybir.dt.float32

B, N, C = 8, 1024, 768
H, D = 12, 64
HID = H * D
KT = C // 128           # 6 feature k-tiles
MT = N // 128           # 8 sequence m-tiles
PAIRS = H // 2          # 6 head pairs
SCALE = D ** -0.5       # 0.125

_cached_nc = None
LABELS = {}

DEFAULT_OPTS = dict(
    pt_bufs=4,
    warmup=5,
    pop_t0=2,           # filler pops per m-iter during attention(0) nh0
    ev_a_pool=False,    # proj pass-a evictions on Pool (vs DVE)
    ev_b_pool=False,    # proj pass-b filler adds on Pool (vs DVE)
    tp_nh1_act=True,   # t5-nh1 transposes on ACT HWDGE queue (vs SP)
    t5pops="C",
)


def _lab(inst, label):
    for cand in (getattr(inst, "ins", None), inst):
        name = getattr(cand, "name", None)
        if isinstance(name, str):
            LABELS[name] = label
            return inst
    return inst


def build_program(**opts):
    o = dict(DEFAULT_OPTS, **opts)
    o.setdefault("debug", False)
    nc = bacc.Bacc(None, target_bir_lowering=False)

    start_d = nc.dram_tensor("start_blk", [KT, 128, N + 256], BF16,
                             kind="ExternalInput")
    wqkr_d = nc.dram_tensor("wqk_rest", [KT, 128, 1280], BF16,
                            kind="ExternalInput")
    wv_d = nc.dram_tensor("wv", [KT, 128, HID], BF16, kind="ExternalInput")
    wp_d = nc.dram_tensor("wp", [KT, 128, C], BF16, kind="ExternalInput")
    ident_d = nc.dram_tensor("ident", [128, 128], BF16, kind="ExternalInput")
    y_d = nc.dram_tensor("y", [N, C], F32, kind="ExternalOutput")
    if o.get("debug"):
        dbg_qkT = nc.dram_tensor("dbg_qkT", [2 * PAIRS, 128, N], F32,
                                 kind="ExternalOutput")
        dbg_v = nc.dram_tensor("dbg_v", [MT, 128, H, D + 1], F32,
                               kind="ExternalOutput")
        dbg_h = nc.dram_tensor("dbg_h", [PAIRS, 128, N], F32,
                               kind="ExternalOutput")
        dbg_hT = nc.dram_tensor("dbg_hT", [PAIRS, 128, N], F32,
                                kind="ExternalOutput")

    with tile.TileContext(nc) as tc:
        with tc.tile_pool(name="persist", bufs=1) as persist, \
             tc.tile_pool(name="pt_pool", bufs=o["pt_bufs"]) as pt_pool, \
             tc.tile_pool(name="nrm_pool", bufs=4) as nrm_pool, \
             tc.tile_pool(name="y_pool", bufs=4) as y_pool, \
             tc.tile_pool(name="ps_s", bufs=2, space="PSUM") as ps_s, \
             tc.tile_pool(name="ps_acc", bufs=1, space="PSUM") as ps_acc, \
             tc.tile_pool(name="ps_scr", bufs=2, space="PSUM") as ps_scr:

            # ---- resident SBUF tiles ----
            xtw = [persist.tile([128, N + 256], BF16, name=f"xtw{k}",
                                tag=f"xtw{k}") for k in range(KT)]
            wqkr = [persist.tile([128, 1280], BF16, name=f"wqkr{k}",
                                 tag=f"wqkr{k}") for k in range(KT)]
            wv = [persist.tile([128, HID], BF16, name=f"wv{k}", tag=f"wv{k}")
                  for k in range(KT)]
            wp = [persist.tile([128, C], BF16, name=f"wp{k}", tag=f"wp{k}")
                  for k in range(KT)]

            for k in range(KT):
                _lab(nc.sync.dma_start(xtw[k][:], start_d[k]), f"dma_xtw{k}")
            for k in range(KT):
                _lab(nc.sync.dma_start(wv[k][:], wv_d[k]), f"dma_wv{k}")
            for k in range(KT):
                _lab(nc.sync.dma_start(wqkr[k][:], wqkr_d[k]), f"dma_wqkr{k}")
            for k in range(KT):
                _lab(nc.sync.dma_start(wp[k][:], wp_d[k]), f"dma_wp{k}")
            ident = persist.tile([128, 128], BF16, name="ident", tag="ident")
            _lab(nc.sync.dma_start(ident[:], ident_d[:, :]), "dma_ident")

            # warm the exp table during the DMA prefix
            warm = persist.tile([1, 8], F32, name="warm", tag="warm")
            nc.gpsimd.memset(warm[:], 0.0)
            _lab(nc.scalar.activation(warm[:], warm[:],
                                      mybir.ActivationFunctionType.Exp),
                 "warm_exp")

            # PE clock warmup: big-N dummy matmuls during the DMA window
            if o["warmup"]:
                dummy = persist.tile([128, 16], BF16, name="dummy",
                                     tag="dummy")
                nc.vector.memset(dummy[:], 0.0)
                dps = ps_scr.tile([128, 512], F32, name="dps", tag="scr")
                d_lhsT = bass.AP(dummy.tensor, dummy.offset,
                                 [[16, 128], [1, 1]])
                d_rhs = bass.AP(dummy.tensor, dummy.offset,
                                [[16, 128], [0, 512]])
                for i in range(o["warmup"]):
                    _lab(nc.tensor.matmul(dps[0:1, :], d_lhsT, d_rhs,
                                          start=True, stop=True),
                         f"warmup{i}")

            # qkT[0..5] = Q^T pair tiles, qkT[6..11] = K^T pair tiles
            qkT = [persist.tile([128, N], BF16, name=f"qkT{t}", tag=f"qkT{t}")
                   for t in range(2 * PAIRS)]
            v_aug = [persist.tile([128, H, D + 1], BF16, name=f"vaug{m}",
                                  tag=f"vaug{m}") for m in range(MT)]
            h_sb = [persist.tile([128, N], BF16, name=f"hsb{t}", tag=f"hsb{t}")
                    for t in range(PAIRS)]
            hT = [persist.tile([128, N], BF16, name=f"hT{t}", tag=f"hT{t}")
                  for t in range(PAIRS)]

            def wqk_src(t, reg, k):
                if t == 0:
                    return xtw[k][:, reg * 128:(reg + 1) * 128]
                off = (t - 1) * 256 + reg * 128
                return wqkr[k][:, off:off + 128]

            def xt(k, sl):
                return xtw[k][:, 256 + sl.start: 256 + sl.stop]

            # ---- startup: pair-0 Q^T and K^T, k-outer over 4 regions ----
            sq = ps_s.tile([128, N], F32, name="sq", tag="s")
            sk = ps_s.tile([128, N], F32, name="sk", tag="s")
            for k in range(KT):
                st, sp = (k == 0), (k == KT - 1)
                for nh in range(2):
                    nsl = slice(nh * 512, (nh + 1) * 512)
                    _lab(nc.tensor.matmul(sq[:, nsl], wqk_src(0, 0, k),
                                          xt(k, nsl), start=st, stop=sp),
                         f"qk0 q nh{nh} k{k}")
                    _lab(nc.tensor.matmul(sk[:, nsl], wqk_src(0, 1, k),
                                          xt(k, nsl), start=st, stop=sp),
                         f"qk0 k nh{nh} k{k}")
            for nh in range(2):
                nsl = slice(nh * 512, (nh + 1) * 512)
                _lab(nc.vector.tensor_copy(qkT[0][:, nsl], sq[:, nsl]),
                     f"ev_qk0q nh{nh}")
                _lab(nc.vector.tensor_copy(qkT[6][:, nsl], sk[:, nsl]),
                     f"ev_qk0k nh{nh}")

            # ---- filler units ----
            def v_unit(m, vh):
                def run():
                    ps = ps_scr.tile([128, 512], F32, name="ps_v", tag="scr")
                    for k in range(KT):
                        _lab(nc.tensor.matmul(
                            ps[:, 0:384],
                            xt(k, slice(m * 128, (m + 1) * 128)),
                            wv[k][:, vh * 384:(vh + 1) * 384],
                            start=(k == 0), stop=(k == KT - 1)),
                            f"v m{m} vh{vh} k{k}")
                    dst = v_aug[m][:, vh * 6:(vh + 1) * 6, 0:D]
                    _lab(nc.vector.tensor_copy(
                        dst, ps[:, 0:384].rearrange("p (h d) -> p h d", d=D)),
                        f"ev_v m{m} vh{vh}")
                    if vh == 1:
                        _lab(nc.gpsimd.memset(v_aug[m][:, :, D:D + 1], 1.0),
                             f"ones m{m}")
                return run

            def qk_unit(t, reg, nh):
                def run():
                    ps = ps_scr.tile([128, 512], F32, name="ps_qk", tag="scr")
                    nsl = slice(nh * 512, (nh + 1) * 512)
                    for k in range(KT):
                        _lab(nc.tensor.matmul(ps[:, 0:512], wqk_src(t, reg, k),
                                              xt(k, nsl),
                                              start=(k == 0),
                                              stop=(k == KT - 1)),
                             f"qk{t} r{reg} nh{nh} k{k}")
                    _lab(nc.vector.tensor_copy(qkT[t + 6 * reg][:, nsl],
                                               ps[:, 0:512]),
                         f"ev_qk{t} r{reg} nh{nh}")
                return run

            y1_sb = [persist.tile([128, C], F32, name=f"y1_{m}", tag=f"y1_{m}")
                     for m in range(MT)]
            y_sbs = {}

            def proj_a(m, reg):
                # partial projection over hid tiles k=0..3 -> SBUF staging
                def run():
                    msl = slice(m * 128, (m + 1) * 128)
                    ps = ps_scr.tile([128, 512], F32, name="ps_ya", tag="scr")
                    for k in range(4):
                        _lab(nc.tensor.matmul(
                            ps[:, 0:384], hT[k][:, msl],
                            wp[k][:, reg * 384:(reg + 1) * 384],
                            start=(k == 0), stop=(k == 3)),
                            f"proja m{m} r{reg} k{k}")
                    csl = slice(reg * 384, (reg + 1) * 384)
                    # Pool keeps the DVE queue clear for normalize
                    ev_eng = nc.gpsimd if o["ev_a_pool"] else nc.vector
                    _lab(ev_eng.tensor_copy(y1_sb[m][:, csl], ps[:, 0:384]),
                         f"ev_ya m{m} r{reg}")
                return run

            def proj_b(m, reg):
                # k=4,5 + add staged partial during eviction, then DMA out
                def run():
                    msl = slice(m * 128, (m + 1) * 128)
                    ps = ps_scr.tile([128, 512], F32, name="ps_yb", tag="scr")
                    for k in (4, 5):
                        _lab(nc.tensor.matmul(
                            ps[:, 0:384], hT[k][:, msl],
                            wp[k][:, reg * 384:(reg + 1) * 384],
                            start=(k == 4), stop=(k == 5)),
                            f"projb m{m} r{reg} k{k}")
                    if m not in y_sbs:
                        y_sbs[m] = y_pool.tile([128, C], F32,
                                               name="y_sb", tag="y")
                    y_sb = y_sbs[m]
                    csl = slice(reg * 384, (reg + 1) * 384)
                    ev_eng = nc.gpsimd if o["ev_b_pool"] else nc.vector
                    _lab(ev_eng.tensor_add(y_sb[:, csl], ps[:, 0:384],
                                           y1_sb[m][:, csl]),
                         f"ev_yb m{m} r{reg}")
                    if reg == 1:
                        _lab(nc.sync.dma_start(y_d[msl, :], y_sb[:]),
                             f"dma_y m{m}")
                return run

            filler = deque()
            for m in range(1, MT):
                for vh in range(2):
                    filler.append((0, v_unit(m, vh)))
            for t in range(1, PAIRS):
                for reg in range(2):
                    for nh in range(2):
                        filler.append((t, qk_unit(t, reg, nh)))
            v_unit(0, 0)()
            v_unit(0, 1)()

            def pop_filler(t, nh, m):
                if t == 0 and nh == 0:
                    budget = o["pop_t0"]
                elif t == 5 and nh == 1:
                    pol = o.get("t5pops", "A")
                    if pol == "A":
                        budget = 2 if (1 <= m <= 4) else 0
                    elif pol == "B":
                        budget = 2 if (m % 2 == 1) else 0
                    else:
                        budget = 2 if (m % 2 == 1) else (1 if m >= 2 else 0)
                elif t >= 4:
                    budget = 1 if (m % 2 == 0) else 0
                else:
                    budget = 1 if (m % 4 == 0) else 0
                while budget > 0 and filler:
                    deadline, unit = filler[0]
                    if deadline > t + 1:
                        break
                    filler.popleft()
                    unit()
                    budget -= 1

            def transpose_pe(t, col0, ncols):
                for i in range(ncols // 128):
                    c = col0 + i * 128
                    ps = ps_scr.tile([128, 512], F32, name="ps_tp", tag="scr")
                    psb = ps.bitcast(BF16)
                    _lab(nc.tensor.transpose(
                        psb[:, 0:128], h_sb[t][:, c:c + 128], ident[:]),
                        f"petp{t} c{c}")
                    # ACT is idle after the last exp; keep DVE free for
                    # the y adds
                    _lab(nc.scalar.copy(hT[t][:, c:c + 128], psb[:, 0:128]),
                         f"ev_tp{t} c{c}")

            def transpose_part(t, col0, ncols, eng=None):
                src = h_sb[t][:, col0:col0 + ncols]
                dst = hT[t][:, col0:col0 + ncols].rearrange(
                    "p (nt n) -> p nt n", n=128)
                _lab((eng or nc.sync).dma_start_transpose(dst, src),
                     f"tp{t} c{col0}")

            def normalize(t, nh, acc, ntl0, ntn):
                arow = acc.ap[0][0]
                recip = nrm_pool.tile([128, 8], F32, name="recip",
                                      tag="recip")
                rrow = recip.ap[0][0]
                rs_in = bass.AP(acc.tensor, acc.offset + D + ntl0 * 65,
                                [[arow, 128], [512, 2], [65, ntn]])
                rs_out = bass.AP(recip.tensor, recip.offset,
                                 [[rrow, 128], [4, 2], [1, ntn]])
                _lab(nc.vector.reciprocal(rs_out, rs_in),
                     f"recip t{t} nh{nh} n{ntl0}")
                h_in = bass.AP(acc.tensor, acc.offset + ntl0 * 65,
                               [[arow, 128], [512, 2], [65, ntn], [1, D]])
                r_in = bass.AP(recip.tensor, recip.offset,
                               [[rrow, 128], [4, 2], [1, ntn], [0, D]])
                hrow = h_sb[t].ap[0][0]
                h_out = bass.AP(h_sb[t].tensor,
                                h_sb[t].offset + nh * 512 + ntl0 * 128,
                                [[hrow, 128], [D, 2], [128, ntn], [1, D]])
                _lab(nc.vector.tensor_mul(h_out, h_in, r_in),
                     f"norm t{t} nh{nh} n{ntl0}")

            # ---- attention ----
            def attn_half(t, nh, split_tail=False):
                qT_t, kT_t = qkT[t], qkT[6 + t]
                nsl = slice(nh * 512, (nh + 1) * 512)
                acc = ps_acc.tile([128, N], F32, name="acc", tag="acc")
                p_tiles = [None] * MT

                # PSUM start=True clears the whole bank's accumulate-valid
                # bits ("first touch replaces, then accumulate"): issue
                # exactly one start per bank (j0/j1 live in separate banks)
                # on the first region of m=0; everything else accumulates.
                def pv(m):
                    for j in range(2):
                        for nt in range(4):
                            off = j * 512 + nt * 65
                            _lab(nc.tensor.matmul(
                                acc[:, off:off + 65],
                                p_tiles[m][:, j * 512 + nt * 128:
                                           j * 512 + (nt + 1) * 128],
                                v_aug[m][:, 2 * t + j, :],
                                start=(m == 0 and nt == 0),
                                stop=(m == MT - 1),
                                skip_group_check=True),
                                f"pv t{t} nh{nh} m{m} j{j} nt{nt}")

                for m in range(MT):
                    msl = slice(m * 128, (m + 1) * 128)
                    s_ps = ps_s.tile([128, N], F32, name="s_ps", tag="s")
                    for j in range(2):
                        psl = slice(j * 64, (j + 1) * 64)
                        _lab(nc.tensor.matmul(s_ps[:, j * 512:(j + 1) * 512],
                                              kT_t[psl, msl], qT_t[psl, nsl],
                                              start=True, stop=True),
                             f"S t{t} nh{nh} m{m} j{j}")
                    p_sb = pt_pool.tile([128, N], BF16, name="p_sb", tag="p")
                    _lab(nc.scalar.activation(p_sb[:], s_ps[:],
                                              mybir.ActivationFunctionType.Exp,
                                              scale=SCALE),
                         f"exp t{t} nh{nh} m{m}")
                    p_tiles[m] = p_sb
                    pop_filler(t, nh, m)
                    if m >= 1:
                        pv(m - 1)
                pv(MT - 1)

                if split_tail:
                    for c in range(2):
                        with tc.high_priority():
                            normalize(t, nh, acc, c * 2, 2)
                        if nh == 1:
                            transpose_pe(t, nh * 512 + c * 256, 256)
                        else:
                            transpose_part(t, nh * 512 + c * 256, 256)
                else:
                    normalize(t, nh, acc, 0, 4)

            for t in range(PAIRS):
                attn_half(t, 0, split_tail=(t == 5))
                if t < 5:
                    transpose_part(t, 0, 512)
                else:
                    for m in range(2):
                        for reg in range(2):
                            filler.append((5, proj_b(m, reg)))
                attn_half(t, 1, split_tail=(t == 5))
                if t < 5:
                    transpose_part(t, 512, 512)
                if t == 3:
                    # hT[0..3] complete after these transposes: stage the
                    # k<4 projection partials as t4/t5 filler
                    for m in range(MT):
                        for reg in range(2):
                            filler.append((5, proj_a(m, reg)))

            # ---- projection tail: m=2..7, accumulators spread across
            # every free PSUM bank so no slot-reuse WAR serializes the k5s
            while filler:
                filler.popleft()[1]()

            def tail_slot(idx):
                kind = ("s", "s", "acc", "s", "s", "scr2")[idx]
                if kind == "s":
                    ps = ps_s.tile([128, N], F32, name="ps_yt", tag="s")
                    return [ps[:, 0:384], ps[:, 512:896]]
                if kind == "acc":
                    ps = ps_acc.tile([128, N], F32, name="ps_ya2", tag="acc")
                    return [ps[:, 0:384], ps[:, 512:896]]
                ps0 = ps_scr.tile([128, 512], F32, name="ps_ys0", tag="scr")
                ps1 = ps_scr.tile([128, 512], F32, name="ps_ys1", tag="scr")
                return [ps0[:, 0:384], ps1[:, 0:384]]

            for i, m in enumerate(range(2, MT)):
                msl = slice(m * 128, (m + 1) * 128)
                regs = tail_slot(i)
                for reg in range(2):
                    for k in (4, 5):
                        _lab(nc.tensor.matmul(
                            regs[reg], hT[k][:, msl],
                            wp[k][:, reg * 384:(reg + 1) * 384],
                            start=(k == 4), stop=(k == 5)),
                            f"projb m{m} r{reg} k{k}")
                y_sb = y_pool.tile([128, C], F32, name="y_sbt", tag="y")
                for reg in range(2):
                    csl = slice(reg * 384, (reg + 1) * 384)
                    if m in (3, 5, 7):
                        # ACT evicts PSUM->SBUF, Pool adds: keeps the DVE
                        # add chain short in the DMA-bound tail
                        stg = y_pool.tile([128, 384], F32, name="ystg",
                                          tag="ystg")
                        _lab(nc.scalar.copy(stg[:], regs[reg]),
                             f"cp_yT m{m} r{reg}")
                        _lab(nc.gpsimd.tensor_add(y_sb[:, csl], stg[:],
                                                  y1_sb[m][:, csl]),
                             f"ev_yT m{m} r{reg}")
                    else:
                        _lab(nc.vector.tensor_add(y_sb[:, csl], regs[reg],
                                                  y1_sb[m][:, csl]),
                             f"ev_yT m{m} r{reg}")
                _lab(nc.sync.dma_start(y_d[msl, :], y_sb[:]), f"dma_yT m{m}")

            if o.get("debug"):
                dbf = persist.tile([128, N], F32, name="dbf", tag="dbf")
                for t in range(2 * PAIRS):
                    nc.vector.tensor_copy(dbf[:], qkT[t][:])
                    nc.sync.dma_start(dbg_qkT[t], dbf[:])
                dbv = persist.tile([128, H * (D + 1)], F32, name="dbv",
                                   tag="dbv")
                for m in range(MT):
                    nc.vector.tensor_copy(
                        dbv[:].rearrange("p (h d) -> p h d", d=D + 1),
                        v_aug[m][:])
                    nc.sync.dma_start(
                        dbg_v[m],
                        dbv[:].rearrange("p (h d) -> p h d", d=D + 1))
                for t in range(PAIRS):
                    nc.vector.tensor_copy(dbf[:], h_sb[t][:])
                    nc.sync.dma_start(dbg_h[t], dbf[:])
                for t in range(PAIRS):
                    nc.vector.tensor_copy(dbf[:], hT[t][:])
                    nc.sync.dma_start(dbg_hT[t], dbf[:])

    nc.compile()
    return nc


def _prep_inputs(inputs):
    import ml_dtypes
    bf16 = ml_dtypes.bfloat16
    x = np.asarray(inputs["x"], dtype=np.float32)
    wqkv = np.asarray(inputs["W_qkv"], dtype=np.float32)
    wproj = np.asarray(inputs["W_proj"], dtype=np.float32)

    wq, wk, wv_np = wqkv[:, :HID], wqkv[:, HID:2 * HID], wqkv[:, 2 * HID:]
    blocks = []
    for t in range(PAIRS):
        blocks.append(wq[:, t * 128:(t + 1) * 128])
        blocks.append(wk[:, t * 128:(t + 1) * 128])
    wqkR = np.concatenate(blocks, axis=1)  # [C, 1536]

    wqk0 = wqkR[:, 0:256].reshape(KT, 128, 256)
    wqk_rest = np.ascontiguousarray(
        wqkR[:, 256:1536].reshape(KT, 128, 1280)).astype(bf16)
    wv_a = np.ascontiguousarray(wv_np.reshape(KT, 128, HID)).astype(bf16)
    wp_a = np.ascontiguousarray(wproj.reshape(KT, 128, C)).astype(bf16)
    ident_a = np.eye(128, dtype=np.float32).astype(bf16)

    per_core = []
    for b in range(B):
        xT = x[b].T.reshape(KT, 128, N)
        start = np.concatenate([wqk0, xT], axis=2).astype(bf16)
        per_core.append({
            "start_blk": np.ascontiguousarray(start),
            "wqk_rest": wqk_rest,
            "wv": wv_a,
            "wp": wp_a,
            "ident": ident_a,
        })
    return per_core


def _run(inputs, trace=False, trace_kwargs=None):
    global _cached_nc
    if _cached_nc is None:
        _cached_nc = build_program()
    nc = _cached_nc

    in_maps = _prep_inputs(inputs)
    kwargs = {}
    if trace:
        kwargs["trace"] = True
        if trace_kwargs:
            kwargs.update(trace_kwargs)
    try:
        res = run_bass_kernel_spmd(nc, in_maps, core_ids=list(range(B)),
                                   **kwargs)
    except Exception:
        res = run_bass_kernel_spmd(nc, in_maps, core_ids=list(range(B)),
                                   **kwargs)
    out = np.stack([np.asarray(r["y"], dtype=np.float32)
                    for r in res.results], axis=0)
    return out, res


def kernel(**inputs):
    out, _ = _run(inputs)
    return out


# revision 3
# speedup vs baseline: 1.0007x; 1.0007x over previous
"""Multi-head attention (B=8, N=1024, H=12, D=64, C=768) on 8 trn2 cores.

Sharding: data-parallel over batch (core b owns x[b]; weights replicated;
no collectives). All matmul operands bf16 (host-cast); PSUM accumulates fp32.

Per-core dataflow:
  warmup:  dummy matmuls ramp the PE clock during the input-DMA window
  phase 1a: qT_t/kT_t [128 x N] = Wqk_block.T @ x^T    (d-major, pair t)
  phase 1b: v_aug[m]  [128 x 12 x 65] = x_m @ Wv (+ ones col per head)
  phase 2 (per head pair t, n-half nh, m-tile):
     s_ps[m,n]   = K^T.T @ Q^T    (two K=64 matmuls, one per head j)
     p_sb        = exp(s_ps/8)    (one [128,1024] ScalarE op, bf16 out)
     acc[n,d+1] += p_chunk.T @ v_aug  (lhsT=P^T chunk, N=65; col 64 = rowsum)
     normalize:   one strided reciprocal + one broadcast tensor_mul -> h_sb
  transpose(t): h_sb[t] [n, hid] -> hT[t] [hid, n] via DMA xbar transpose
  phase 3: y[m] = hT.T @ W_proj; m=0..3 interleaved into attention(5) nh1

Emission interleaves v-tile / next-pair qk / proj matmuls into the attention
m-loop (filler deque) so PE stays fed during exp latency.
"""
from collections import deque

import numpy as np

import concourse.bass as bass
import concourse.mybir as mybir
import concourse.tile as tile
from concourse import bacc
from concourse.bass_utils import run_bass_kernel_spmd

BF16 = mybir.dt.bfloat16
F32 = mybir.dt.float32

B, N, C = 8, 1024, 768
H, D = 12, 64
HID = H * D
KT = C // 128           # 6 feature k-tiles
MT = N // 128           # 8 sequence m-tiles
PAIRS = H // 2          # 6 head pairs
SCALE = D ** -0.5       # 0.125

_cached_nc = None
LABELS = {}

DEFAULT_OPTS = dict(
    pt_bufs=4,
    warmup=5,
    pop_t0=2,           # filler pops per m-iter during attention(0) nh0
    ev_a_pool=False,    # proj pass-a evictions on Pool (vs DVE)
    ev_b_pool=False,    # proj pass-b filler adds on Pool (vs DVE)
    tp_nh1_act=True,   # t5-nh1 transposes on ACT HWDGE queue (vs SP)
    t5pops="C",
)


def _lab(inst, label):
    for cand in (getattr(inst, "ins", None), inst):
        name = getattr(cand, "name", None)
        if isinstance(name, str):
            LABELS[name] = label
            return inst
    return inst


def build_program(**opts):
    o = dict(DEFAULT_OPTS, **opts)
    o.setdefault("debug", False)
    nc = bacc.Bacc(None, target_bir_lowering=False)

    start_d = nc.dram_tensor("start_blk", [KT, 128, N + 256], BF16,
                             kind="ExternalInput")
    wqkr_d = nc.dram_tensor("wqk_rest", [KT, 128, 1280], BF16,
                            kind="ExternalInput")
    wv_d = nc.dram_tensor("wv", [KT, 128, HID], BF16, kind="ExternalInput")
    wp_d = nc.dram_tensor("wp", [KT, 128, C], BF16, kind="ExternalInput")
    ident_d = nc.dram_tensor("ident", [128, 128], BF16, kind="ExternalInput")
    y_d = nc.dram_tensor("y", [N, C], F32, kind="ExternalOutput")
    if o.get("debug"):
        dbg_qkT = nc.dram_tensor("dbg_qkT", [2 * PAIRS, 128, N], F32,
                                 kind="ExternalOutput")
        dbg_v = nc.dram_tensor("dbg_v", [MT, 128, H, D + 1], F32,
                               kind="ExternalOutput")
        dbg_h = nc.dram_tensor("dbg_h", [PAIRS, 128, N], F32,
                               kind="ExternalOutput")
        dbg_hT = nc.dram_tensor("dbg_hT", [PAIRS, 128, N], F32,
                                kind="ExternalOutput")

    with tile.TileContext(nc) as tc:
        with tc.tile_pool(name="persist", bufs=1) as persist, \
             tc.tile_pool(name="pt_pool", bufs=o["pt_bufs"]) as pt_pool, \
             tc.tile_pool(name="nrm_pool", bufs=4) as nrm_pool, \
             tc.tile_pool(name="y_pool", bufs=4) as y_pool, \
             tc.tile_pool(name="ps_s", bufs=2, space="PSUM") as ps_s, \
             tc.tile_pool(name="ps_acc", bufs=1, space="PSUM") as ps_acc, \
             tc.tile_pool(name="ps_scr", bufs=2, space="PSUM") as ps_scr:

            # ---- resident SBUF tiles ----
            xtw = [persist.tile([128, N + 256], BF16, name=f"xtw{k}",
                                tag=f"xtw{k}") for k in range(KT)]
            wqkr = [persist.tile([128, 1280], BF16, name=f"wqkr{k}",
                                 tag=f"wqkr{k}") for k in range(KT)]
            wv = [persist.tile([128, HID], BF16, name=f"wv{k}", tag=f"wv{k}")
                  for k in range(KT)]
            wp = [persist.tile([128, C], BF16, name=f"wp{k}", tag=f"wp{k}")
                  for k in range(KT)]

            for k in range(KT):
                _lab(nc.sync.dma_start(xtw[k][:], start_d[k]), f"dma_xtw{k}")
            for k in range(KT):
                _lab(nc.sync.dma_start(wv[k][:], wv_d[k]), f"dma_wv{k}")
            for k in range(KT):
                _lab(nc.sync.dma_start(wqkr[k][:], wqkr_d[k]), f"dma_wqkr{k}")
            for k in range(KT):
                _lab(nc.sync.dma_start(wp[k][:], wp_d[k]), f"dma_wp{k}")
            ident = persist.tile([128, 128], BF16, name="ident", tag="ident")
            _lab(nc.sync.dma_start(ident[:], ident_d[:, :]), "dma_ident")

            # warm the exp table during the DMA prefix
            warm = persist.tile([1, 8], F32, name="warm", tag="warm")
            nc.gpsimd.memset(warm[:], 0.0)
            _lab(nc.scalar.activation(warm[:], warm[:],
                                      mybir.ActivationFunctionType.Exp),
                 "warm_exp")

            # PE clock warmup: big-N dummy matmuls during the DMA window
            if o["warmup"]:
                dummy = persist.tile([128, 16], BF16, name="dummy",
                                     tag="dummy")
                nc.vector.memset(dummy[:], 0.0)
                dps = ps_scr.tile([128, 512], F32, name="dps", tag="scr")
                d_lhsT = bass.AP(dummy.tensor, dummy.offset,
                                 [[16, 128], [1, 1]])
                d_rhs = bass.AP(dummy.tensor, dummy.offset,
                                [[16, 128], [0, 512]])
                for i in range(o["warmup"]):
                    _lab(nc.tensor.matmul(dps[0:1, :], d_lhsT, d_rhs,
                                          start=True, stop=True),
                         f"warmup{i}")

            # qkT[0..5] = Q^T pair tiles, qkT[6..11] = K^T pair tiles
            qkT = [persist.tile([128, N], BF16, name=f"qkT{t}", tag=f"qkT{t}")
                   for t in range(2 * PAIRS)]
            v_aug = [persist.tile([128, H, D + 1], BF16, name=f"vaug{m}",
                                  tag=f"vaug{m}") for m in range(MT)]
            h_sb = [persist.tile([128, N], BF16, name=f"hsb{t}", tag=f"hsb{t}")
                    for t in range(PAIRS)]
            hT = [persist.tile([128, N], BF16, name=f"hT{t}", tag=f"hT{t}")
                  for t in range(PAIRS)]

            def wqk_src(t, reg, k):
                if t == 0:
                    return xtw[k][:, reg * 128:(reg + 1) * 128]
                off = (t - 1) * 256 + reg * 128
                return wqkr[k][:, off:off + 128]

            def xt(k, sl):
                return xtw[k][:, 256 + sl.start: 256 + sl.stop]

            # ---- startup: pair-0 Q^T and K^T, k-outer over 4 regions ----
            sq = ps_s.tile([128, N], F32, name="sq", tag="s")
            sk = ps_s.tile([128, N], F32, name="sk", tag="s")
            for k in range(KT):
                st, sp = (k == 0), (k == KT - 1)
                for nh in range(2):
                    nsl = slice(nh * 512, (nh + 1) * 512)
                    _lab(nc.tensor.matmul(sq[:, nsl], wqk_src(0, 0, k),
                                          xt(k, nsl), start=st, stop=sp),
                         f"qk0 q nh{nh} k{k}")
                    _lab(nc.tensor.matmul(sk[:, nsl], wqk_src(0, 1, k),
                                          xt(k, nsl), start=st, stop=sp),
                         f"qk0 k nh{nh} k{k}")
            for nh in range(2):
                nsl = slice(nh * 512, (nh + 1) * 512)
                _lab(nc.vector.tensor_copy(qkT[0][:, nsl], sq[:, nsl]),
                     f"ev_qk0q nh{nh}")
                _lab(nc.vector.tensor_copy(qkT[6][:, nsl], sk[:, nsl]),
                     f"ev_qk0k nh{nh}")

            # ---- filler units ----
            def v_unit(m, vh):
                def run():
                    ps = ps_scr.tile([128, 512], F32, name="ps_v", tag="scr")
                    for k in range(KT):
                        _lab(nc.tensor.matmul(
                            ps[:, 0:384],
                            xt(k, slice(m * 128, (m + 1) * 128)),
                            wv[k][:, vh * 384:(vh + 1) * 384],
                            start=(k == 0), stop=(k == KT - 1)),
                            f"v m{m} vh{vh} k{k}")
                    dst = v_aug[m][:, vh * 6:(vh + 1) * 6, 0:D]
                    _lab(nc.vector.tensor_copy(
                        dst, ps[:, 0:384].rearrange("p (h d) -> p h d", d=D)),
                        f"ev_v m{m} vh{vh}")
                    if vh == 1:
                        _lab(nc.gpsimd.memset(v_aug[m][:, :, D:D + 1], 1.0),
                             f"ones m{m}")
                return run

            def qk_unit(t, reg, nh):
                def run():
                    ps = ps_scr.tile([128, 512], F32, name="ps_qk", tag="scr")
                    nsl = slice(nh * 512, (nh + 1) * 512)
                    for k in range(KT):
                        _lab(nc.tensor.matmul(ps[:, 0:512], wqk_src(t, reg, k),
                                              xt(k, nsl),
                                              start=(k == 0),
                                              stop=(k == KT - 1)),
                             f"qk{t} r{reg} nh{nh} k{k}")
                    _lab(nc.vector.tensor_copy(qkT[t + 6 * reg][:, nsl],
                                               ps[:, 0:512]),
                         f"ev_qk{t} r{reg} nh{nh}")
                return run

            y1_sb = [persist.tile([128, C], F32, name=f"y1_{m}", tag=f"y1_{m}")
                     for m in range(MT)]
            y_sbs = {}

            def proj_a(m, reg):
                # partial projection over hid tiles k=0..3 -> SBUF staging
                def run():
                    msl = slice(m * 128, (m + 1) * 128)
                    ps = ps_scr.tile([128, 512], F32, name="ps_ya", tag="scr")
                    for k in range(4):
                        _lab(nc.tensor.matmul(
                            ps[:, 0:384], hT[k][:, msl],
                            wp[k][:, reg * 384:(reg + 1) * 384],
                            start=(k == 0), stop=(k == 3)),
                            f"proja m{m} r{reg} k{k}")
                    csl = slice(reg * 384, (reg + 1) * 384)
                    # Pool keeps the DVE queue clear for normalize
                    ev_eng = nc.gpsimd if o["ev_a_pool"] else nc.vector
                    _lab(ev_eng.tensor_copy(y1_sb[m][:, csl], ps[:, 0:384]),
                         f"ev_ya m{m} r{reg}")
                return run

            def proj_b(m, reg):
                # k=4,5 + add staged partial during eviction, then DMA out
                def run():
                    msl = slice(m * 128, (m + 1) * 128)
                    ps = ps_scr.tile([128, 512], F32, name="ps_yb", tag="scr")
                    for k in (4, 5):
                        _lab(nc.tensor.matmul(
                            ps[:, 0:384], hT[k][:, msl],
                            wp[k][:, reg * 384:(reg + 1) * 384],
                            start=(k == 4), stop=(k == 5)),
                            f"projb m{m} r{reg} k{k}")
                    if m not in y_sbs:
                        y_sbs[m] = y_pool.tile([128, C], F32,
                                               name="y_sb", tag="y")
                    y_sb = y_sbs[m]
                    csl = slice(reg * 384, (reg + 1) * 384)
                    ev_eng = nc.gpsimd if o["ev_b_pool"] else nc.vector
                    _lab(ev_eng.tensor_add(y_sb[:, csl], ps[:, 0:384],
                                           y1_sb[m][:, csl]),
                         f"ev_yb m{m} r{reg}")
                    if reg == 1:
                        _lab(nc.sync.dma_start(y_d[msl, :], y_sb[:]),
                             f"dma_y m{m}")
                return run

            filler = deque()
            for m in range(1, MT):
                for vh in range(2):
                    filler.append((0, v_unit(m, vh)))
            for t in range(1, PAIRS):
                for reg in range(2):
                    for nh in range(2):
                        filler.append((t, qk_unit(t, reg, nh)))
            v_unit(0, 0)()
            v_unit(0, 1)()

            def pop_filler(t, nh, m):
                if t == 0 and nh == 0:
                    budget = o["pop_t0"]
                elif t == 5 and nh == 1:
                    pol = o.get("t5pops", "A")
                    if pol == "A":
                        budget = 2 if (1 <= m <= 4) else 0
                    elif pol == "B":
                        budget = 2 if (m % 2 == 1) else 0
                    else:
                        budget = 2 if (m % 2 == 1) else (1 if m >= 2 else 0)
                elif t >= 4:
                    budget = 1 if (m % 2 == 0) else 0
                else:
                    budget = 1 if (m % 4 == 0) else 0
                while budget > 0 and filler:
                    deadline, unit = filler[0]
                    if deadline > t + 1:
                        break
                    filler.popleft()
                    unit()
                    budget -= 1

            def transpose_pe(t, col0, ncols):
                for i in range(ncols // 128):
                    c = col0 + i * 128
                    ps = ps_scr.tile([128, 512], F32, name="ps_tp", tag="scr")
                    psb = ps.bitcast(BF16)
                    _lab(nc.tensor.transpose(
                        psb[:, 0:128], h_sb[t][:, c:c + 128], ident[:]),
                        f"petp{t} c{c}")
                    # ACT is idle after the last exp; keep DVE free for
                    # the y adds
                    _lab(nc.scalar.copy(hT[t][:, c:c + 128], psb[:, 0:128]),
                         f"ev_tp{t} c{c}")

            def transpose_part(t, col0, ncols, eng=None):
                src = h_sb[t][:, col0:col0 + ncols]
                dst = hT[t][:, col0:col0 + ncols].rearrange(
                    "p (nt n) -> p nt n", n=128)
                _lab((eng or nc.sync).dma_start_transpose(dst, src),
                     f"tp{t} c{col0}")

            def normalize(t, nh, acc, ntl0, ntn):
                arow = acc.ap[0][0]
                recip = nrm_pool.tile([128, 8], F32, name="recip",
                                      tag="recip")
                rrow = recip.ap[0][0]
                rs_in = bass.AP(acc.tensor, acc.offset + D + ntl0 * 65,
                                [[arow, 128], [512, 2], [65, ntn]])
                rs_out = bass.AP(recip.tensor, recip.offset,
                                 [[rrow, 128], [4, 2], [1, ntn]])
                _lab(nc.vector.reciprocal(rs_out, rs_in),
                     f"recip t{t} nh{nh} n{ntl0}")
                h_in = bass.AP(acc.tensor, acc.offset + ntl0 * 65,
                               [[arow, 128], [512, 2], [65, ntn], [1, D]])
                r_in = bass.AP(recip.tensor, recip.offset,
                               [[rrow, 128], [4, 2], [1, ntn], [0, D]])
                hrow = h_sb[t].ap[0][0]
                h_out = bass.AP(h_sb[t].tensor,
                                h_sb[t].offset + nh * 512 + ntl0 * 128,
                                [[hrow, 128], [D, 2], [128, ntn], [1, D]])
                _lab(nc.vector.tensor_mul(h_out, h_in, r_in),
                     f"norm t{t} nh{nh} n{ntl0}")

            # ---- attention ----
            def attn_half(t, nh, split_tail=False):
                qT_t, kT_t = qkT[t], qkT[6 + t]
                nsl = slice(nh * 512, (nh + 1) * 512)
                acc = ps_acc.tile([128, N], F32, name="acc", tag="acc")
                p_tiles = [None] * MT

                # PSUM start=True clears the whole bank's accumulate-valid
                # bits ("first touch replaces, then accumulate"): issue
                # exactly one start per bank (j0/j1 live in separate banks)
                # on the first region of m=0; everything else accumulates.
                def pv(m):
                    for j in range(2):
                        for nt in range(4):
                            off = j * 512 + nt * 65
                            _lab(nc.tensor.matmul(
                                acc[:, off:off + 65],
                                p_tiles[m][:, j * 512 + nt * 128:
                                           j * 512 + (nt + 1) * 128],
                                v_aug[m][:, 2 * t + j, :],
                                start=(m == 0 and nt == 0),
                                stop=(m == MT - 1),
                                skip_group_check=True),
                                f"pv t{t} nh{nh} m{m} j{j} nt{nt}")

                for m in range(MT):
                    msl = slice(m * 128, (m + 1) * 128)
                    s_ps = ps_s.tile([128, N], F32, name="s_ps", tag="s")
                    for j in range(2):
                        psl = slice(j * 64, (j + 1) * 64)
                        _lab(nc.tensor.matmul(s_ps[:, j * 512:(j + 1) * 512],
                                              kT_t[psl, msl], qT_t[psl, nsl],
                                              start=True, stop=True),
                             f"S t{t} nh{nh} m{m} j{j}")
                    p_sb = pt_pool.tile([128, N], BF16, name="p_sb", tag="p")
                    _lab(nc.scalar.activation(p_sb[:], s_ps[:],
                                              mybir.ActivationFunctionType.Exp,
                                              scale=SCALE),
                         f"exp t{t} nh{nh} m{m}")
                    p_tiles[m] = p_sb
                    pop_filler(t, nh, m)
                    if m >= 1:
                        pv(m - 1)
                pv(MT - 1)

                if split_tail:
                    for c in range(2):
                        with tc.high_priority():
                            normalize(t, nh, acc, c * 2, 2)
                        if nh == 1:
                            transpose_pe(t, nh * 512 + c * 256, 256)
                        else:
                            transpose_part(t, nh * 512 + c * 256, 256)
                else:
                    normalize(t, nh, acc, 0, 4)

            for t in range(PAIRS):
                attn_half(t, 0, split_tail=(t == 5))
                if t < 5:
                    transpose_part(t, 0, 512)
                else:
                    for m in range(2):
                        for reg in range(2):
                            filler.append((5, proj_b(m, reg)))
                attn_half(t, 1, split_tail=(t == 5))
                if t < 5:
                    transpose_part(t, 512, 512)
                if t == 3:
                    # hT[0..3] complete after these transposes: stage the
                    # k<4 projection partials as t4/t5 filler
                    for m in range(MT):
                        for reg in range(2):
                            filler.append((5, proj_a(m, reg)))

            # ---- projection tail: m=2..7, accumulators spread across
            # every free PSUM bank so no slot-reuse WAR serializes the k5s
            while filler:
                filler.popleft()[1]()

            def tail_slot(idx):
                kind = ("s", "s", "acc", "s", "s", "scr2")[idx]
                if kind == "s":
                    ps = ps_s.tile([128, N], F32, name="ps_yt", tag="s")
                    return [ps[:, 0:384], ps[:, 512:896]]
                if kind == "acc":
                    ps = ps_acc.tile([128, N], F32, name="ps_ya2", tag="acc")
                    return [ps[:, 0:384], ps[:, 512:896]]
                ps0 = ps_scr.tile([128, 512], F32, name="ps_ys0", tag="scr")
                ps1 = ps_scr.tile([128, 512], F32, name="ps_ys1", tag="scr")
                return [ps0[:, 0:384], ps1[:, 0:384]]

            for i, m in enumerate(range(2, MT)):
                msl = slice(m * 128, (m + 1) * 128)
                regs = tail_slot(i)
                for reg in range(2):
                    for k in (4, 5):
                        _lab(nc.tensor.matmul(
                            regs[reg], hT[k][:, msl],
                            wp[k][:, reg * 384:(reg + 1) * 384],
                            start=(k == 4), stop=(k == 5)),
                            f"projb m{m} r{reg} k{k}")
                y_sb = y_pool.tile([128, C], F32, name="y_sbt", tag="y")
                for reg in range(2):
                    csl = slice(reg * 384, (reg + 1) * 384)
                    if m in o.get("tail_pool_ms", (3,)):
                        # ACT evicts PSUM->SBUF, Pool adds: keeps the DVE
                        # add chain short in the DMA-bound tail
                        stg = y_pool.tile([128, 384], F32, name="ystg",
                                          tag="ystg")
                        _lab(nc.scalar.copy(stg[:], regs[reg]),
                             f"cp_yT m{m} r{reg}")
                        _lab(nc.gpsimd.tensor_add(y_sb[:, csl], stg[:],
                                                  y1_sb[m][:, csl]),
                             f"ev_yT m{m} r{reg}")
                    else:
                        _lab(nc.vector.tensor_add(y_sb[:, csl], regs[reg],
                                                  y1_sb[m][:, csl]),
                             f"ev_yT m{m} r{reg}")
                if m in o.get("tail_split_dma", (5, 6, 7)):
                    for reg in range(2):
                        csl = slice(reg * 384, (reg + 1) * 384)
                        _lab(nc.sync.dma_start(y_d[msl, csl], y_sb[:, csl]),
                             f"dma_yT m{m} r{reg}")
                else:
                    _lab(nc.sync.dma_start(y_d[msl, :], y_sb[:]),
                         f"dma_yT m{m}")

            if o.get("debug"):
                dbf = persist.tile([128, N], F32, name="dbf", tag="dbf")
                for t in range(2 * PAIRS):
                    nc.vector.tensor_copy(dbf[:], qkT[t][:])
                    nc.sync.dma_start(dbg_qkT[t], dbf[:])
                dbv = persist.tile([128, H * (D + 1)], F32, name="dbv",
                                   tag="dbv")
                for m in range(MT):
                    nc.vector.tensor_copy(
                        dbv[:].rearrange("p (h d) -> p h d", d=D + 1),
                        v_aug[m][:])
                    nc.sync.dma_start(
                        dbg_v[m],
                        dbv[:].rearrange("p (h d) -> p h d", d=D + 1))
                for t in range(PAIRS):
                    nc.vector.tensor_copy(dbf[:], h_sb[t][:])
                    nc.sync.dma_start(dbg_h[t], dbf[:])
                for t in range(PAIRS):
                    nc.vector.tensor_copy(dbf[:], hT[t][:])
                    nc.sync.dma_start(dbg_hT[t], dbf[:])

    nc.compile()
    return nc


def _prep_inputs(inputs):
    import ml_dtypes
    bf16 = ml_dtypes.bfloat16
    x = np.asarray(inputs["x"], dtype=np.float32)
    wqkv = np.asarray(inputs["W_qkv"], dtype=np.float32)
    wproj = np.asarray(inputs["W_proj"], dtype=np.float32)

    wq, wk, wv_np = wqkv[:, :HID], wqkv[:, HID:2 * HID], wqkv[:, 2 * HID:]
    blocks = []
    for t in range(PAIRS):
        blocks.append(wq[:, t * 128:(t + 1) * 128])
        blocks.append(wk[:, t * 128:(t + 1) * 128])
    wqkR = np.concatenate(blocks, axis=1)  # [C, 1536]

    wqk0 = wqkR[:, 0:256].reshape(KT, 128, 256)
    wqk_rest = np.ascontiguousarray(
        wqkR[:, 256:1536].reshape(KT, 128, 1280)).astype(bf16)
    wv_a = np.ascontiguousarray(wv_np.reshape(KT, 128, HID)).astype(bf16)
    wp_a = np.ascontiguousarray(wproj.reshape(KT, 128, C)).astype(bf16)
    ident_a = np.eye(128, dtype=np.float32).astype(bf16)

    per_core = []
    for b in range(B):
        xT = x[b].T.reshape(KT, 128, N)
        start = np.concatenate([wqk0, xT], axis=2).astype(bf16)
        per_core.append({
            "start_blk": np.ascontiguousarray(start),
            "wqk_rest": wqk_rest,
            "wv": wv_a,
            "wp": wp_a,
            "ident": ident_a,
        })
    return per_core


def _run(inputs, trace=False, trace_kwargs=None):
    global _cached_nc
    if _cached_nc is None:
        _cached_nc = build_program()
    nc = _cached_nc

    in_maps = _prep_inputs(inputs)
    kwargs = {}
    if trace:
        kwargs["trace"] = True
        if trace_kwargs:
            kwargs.update(trace_kwargs)
    try:
        res = run_bass_kernel_spmd(nc, in_maps, core_ids=list(range(B)),
                                   **kwargs)
    except Exception:
        res = run_bass_kernel_spmd(nc, in_maps, core_ids=list(range(B)),
                                   **kwargs)
    out = np.stack([np.asarray(r["y"], dtype=np.float32)
                    for r in res.results], axis=0)
    return out, res


def kernel(**inputs):
    out, _ = _run(inputs)
    return out
